# revision 60
# baseline (speedup 1.0000x reference)
"""BinaryTreeLSTM on 8 Trainium2 NeuronCores (Bass/Tile).

Sharding: each core owns a contiguous subtree of 4096 leaves and reduces
it through the internal levels down to 1024 nodes per core (87.5% of
FLOPs, fully pipelined on device with no flushes). The remaining global
levels (8192 nodes down to the root) are latency-bound on device —
each costs a serial matmul+activation chain plus a pipeline flush for a
shrinking node count — so they are finished on the host with BLAS from
the per-core level-1024 h/c. This also removes the AllGather collective
entirely. Gate weights are replicated across cores.

Per-level node arrays are stored in bit-reversed node order on device so
each level's left/right children are the contiguous halves of the child
level; the host un-permutes when reassembling the natural level-order
output.

Device layout: hidden dim (256 = 2 chunks of 128) on the SBUF partition
axis, nodes on the free axis. Matmuls default to bf16 operands (fp32
PSUM accumulation; ~1e-3 output error) which halves input DMA and
enables fast weight loads; set TRNK_MM_DTYPE=float32r for the fp32r
path. Gate math is fp32 on the Scalar/Vector engines with the per-gate
bias folded into the activation instruction. Inputs stream in over both
hardware-DGE queues (Scalar + Sync) so descriptor generation for the
first-needed pieces overlaps.
"""

import os
import sys

import numpy as np

sys.path.insert(0, "/opt/trn_rl_repo")

HIDDEN = 256
NCORES = 8
CH = 512        # node-chunk (PSUM bank / fp32 moving-operand limit)
CUT = 1024      # smallest per-core level computed on device

# exposed for test harnesses
LAST_RESULTS = None
LAST_EXEC_NS = None
LAST_OPS = None


def _revperm(n):
    bits = n.bit_length() - 1
    r = np.arange(n)
    out = np.zeros(n, np.int64)
    for b in range(bits):
        out |= ((r >> b) & 1) << (bits - 1 - b)
    return out


def _w_tile_index(src, g, kc, hc):
    return ((src * 4 + g) * 2 + kc) * 2 + hc


def _round_fp32r(a):
    """Round fp32 values to the PE's fp32r format (1+8+11 bits, RNE)."""
    bits = np.ascontiguousarray(a, np.float32).view(np.uint32)
    odd = (bits >> np.uint32(12)) & np.uint32(1)
    bits = bits + np.uint32(0x7FF) + odd
    bits &= np.uint32(0xFFFFF000)
    return bits.view(np.float32)


def _pack_weights(Wx, Wl, Wr):
    # lhsT tile for (src, g, kc, hc): [p(contraction), m(out)] = W[g, hc*128+m, kc*128+p]
    tiles = []
    for W in (Wx, Wl, Wr):
        W4 = W.reshape(4, 2, 128, 2, 128)           # [g, hc, m, kc, p]
        tiles.append(W4.transpose(0, 3, 1, 4, 2))    # [g, kc, hc, p, m]
    allw = np.stack(tiles)                            # [3, 4, 2, 2, 128, 128]
    # -> [p, (s,g,kc,hc), m]
    blob = np.ascontiguousarray(allw.transpose(4, 0, 1, 2, 3, 5).reshape(128, 48, 128))
    return blob.astype(np.float32)


def _build_program(LPC, matmul_dtype_name="bfloat16"):
    from concourse import bacc, mybir, tile

    f32 = mybir.dt.float32
    mmdt = getattr(mybir.dt, matmul_dtype_name)
    bf16 = matmul_dtype_name == "bfloat16"
    sdt = mmdt if bf16 else f32
    AF = mybir.ActivationFunctionType

    sizes = []
    n = LPC
    while n >= CUT:
        sizes.append(n)
        n //= 2
    offs = np.concatenate([[0], np.cumsum(sizes)]).astype(int)
    TOT = int(offs[-1])
    NCHUNK = LPC // CH

    nc = bacc.Bacc("TRN2", target_bir_lowering=False, debug=False,
                   num_devices=NCORES)

    x_d = nc.dram_tensor("x", [128, NCHUNK // 2, 2, 2 * CH], mmdt,
                         kind="ExternalInput").ap()
    wt_d = nc.dram_tensor("wt", [128, 48, 128], mmdt, kind="ExternalInput").ap()
    bias_d = nc.dram_tensor("bias", [128, 8], f32, kind="ExternalInput").ap()
    out_d = nc.dram_tensor("out", [2, 128, TOT], sdt, kind="ExternalOutput").ap()
    cend_d = nc.dram_tensor("cend", [2, 128, CUT], f32, kind="ExternalOutput").ap()

    with tile.TileContext(nc) as tc:
        with tc.tile_pool(name="pp", bufs=1) as pp, \
             tc.tile_pool(name="zp", bufs=3, space="PSUM") as zp, \
             tc.tile_pool(name="gp", bufs=3) as gp:
            w_sb = pp.tile([128, 48, 128], mmdt, name="w_sb")
            bias_sb = pp.tile([128, 8], f32, name="bias_sb")
            hA = pp.tile([128, 2, LPC], sdt, name="hA")
            cA = pp.tile([128, 2, LPC], f32, name="cA")
            hB = pp.tile([128, 2, LPC // 2], sdt, name="hB")
            cB = pp.tile([128, 2, LPC // 2], f32, name="cB")
            x_sb = [pp.tile([128, 2, 2 * CH], mmdt, name=f"x_sb{ch}")
                    for ch in range(NCHUNK // 2)]

            # Inputs split across both HW-DGE queues (one queue per engine —
            # same-engine DMAs serialize): bias + leaf weights on Scalar,
            # x chunks on Sync, so the first matmul's inputs land early.
            nc.scalar.dma_start(out=bias_sb[:], in_=bias_d[:])
            nc.scalar.dma_start(out=w_sb[:, 0:16, :], in_=wt_d[:, 0:16, :])
            nc.scalar.dma_start(out=x_sb[1][:], in_=x_d[:, 1])
            nc.scalar.dma_start(out=w_sb[:, 16:48, :], in_=wt_d[:, 16:48, :])
            for ch in (0, 2, 3):
                nc.sync.dma_start(out=x_sb[ch][:], in_=x_d[:, ch])

            if bf16:
                cast_rhs = lambda ap: ap  # noqa: E731
            else:
                cast_rhs = lambda ap: ap.bitcast(mmdt)  # noqa: E731

            def mm(w_idx, rhs_ap, zt, start, stop):
                nc.tensor.matmul(zt, w_sb[:, w_idx, :], cast_rhs(rhs_ap),
                                 start=start, stop=stop)

            def unit_internal(n, ch, h_src, c_src, h_dst, c_dst, out_off,
                              c_out=False):
                """One chunk of an internal level -> (stage1, stage2)."""
                nchunks = max(1, n // CH)
                m = min(n, CH)
                lsl = slice(ch * m, (ch + 1) * m)
                rsl = slice(n + ch * m, n + (ch + 1) * m)
                dsl = slice(ch * m, (ch + 1) * m)
                i_t = gp.tile([128, 2, CH], f32, name="i_t")
                f_t = gp.tile([128, 2, CH], f32, name="f_t")
                o_t = gp.tile([128, 2, CH], f32, name="o_t")
                u_t = gp.tile([128, 2, CH], f32, name="u_t")
                s_t = gp.tile([128, 2, CH], f32, name="s_t")
                gates = {0: i_t, 1: f_t, 2: o_t, 3: u_t}

                def s1():
                    for hc in range(2):
                        nc.gpsimd.tensor_add(s_t[:, hc, :m],
                                             c_src[:, hc, lsl],
                                             c_src[:, hc, rsl])
                    for hc in range(2):
                        for g in (0, 3, 1, 2):
                            zt = zp.tile([128, 2 * CH], f32, name="zt")
                            mm(_w_tile_index(1, g, 0, hc), h_src[:, 0, lsl],
                               zt[:, :m], True, False)
                            mm(_w_tile_index(1, g, 1, hc), h_src[:, 1, lsl],
                               zt[:, :m], False, False)
                            mm(_w_tile_index(2, g, 0, hc), h_src[:, 0, rsl],
                               zt[:, :m], False, False)
                            mm(_w_tile_index(2, g, 1, hc), h_src[:, 1, rsl],
                               zt[:, :m], False, True)
                            func = AF.Tanh if g == 3 else AF.Sigmoid
                            nc.scalar.activation(
                                out=gates[g][:, hc, :m], in_=zt[:, :m],
                                func=func,
                                bias=bias_sb[:, g * 2 + hc:g * 2 + hc + 1])

                def s2():
                    for hc in range(2):
                        nc.vector.tensor_mul(u_t[:, hc, :m], i_t[:, hc, :m],
                                             u_t[:, hc, :m])
                        nc.vector.tensor_mul(s_t[:, hc, :m], f_t[:, hc, :m],
                                             s_t[:, hc, :m])
                        nc.vector.tensor_add(c_dst[:, hc, dsl],
                                             u_t[:, hc, :m], s_t[:, hc, :m])
                        nc.scalar.activation(out=i_t[:, hc, :m],
                                             in_=c_dst[:, hc, dsl],
                                             func=AF.Tanh)
                        nc.vector.tensor_mul(h_dst[:, hc, dsl],
                                             o_t[:, hc, :m], i_t[:, hc, :m])
                    if ch == nchunks - 1:
                        for hc in range(2):
                            nc.sync.dma_start(
                                out=out_d[hc, :, out_off:out_off + n],
                                in_=h_dst[:, hc, :n])
                        if c_out:
                            for hc in range(2):
                                nc.sync.dma_start(
                                    out=cend_d[hc, :, :],
                                    in_=c_dst[:, hc, :n])
                return s1, s2

            def unit_leaves(ch):
                # 1024-node super-chunk: z spans 2 PSUM banks (matmuls stay
                # within one bank each) so a single 1024-column activation
                # with one bias covers the whole chunk per (g, hc) —
                # halves the Scalar engine's per-instruction overhead
                LCH = 2 * CH
                nsl = slice(ch * LCH, (ch + 1) * LCH)
                xc_t = x_sb[ch]
                i_t = gp.tile([128, 2, LCH], sdt, name="li_t")
                o_t = gp.tile([128, 2, LCH], sdt, name="lo_t")
                u_t = gp.tile([128, 2, LCH], sdt, name="lu_t")
                lgates = {0: i_t, 2: o_t, 3: u_t}

                def s1():
                    for hc in range(2):
                        for g in (0, 3, 2):
                            zt = zp.tile([128, LCH], f32, name="zt")
                            for half in (0, CH):
                                hsl = slice(half, half + CH)
                                mm(_w_tile_index(0, g, 0, hc),
                                   xc_t[:, 0, hsl], zt[:, hsl], True, False)
                                mm(_w_tile_index(0, g, 1, hc),
                                   xc_t[:, 1, hsl], zt[:, hsl], False, True)
                            func = AF.Tanh if g == 3 else AF.Sigmoid
                            nc.scalar.activation(
                                out=lgates[g][:, hc, :], in_=zt[:], func=func,
                                bias=bias_sb[:, g * 2 + hc:g * 2 + hc + 1])

                def s2():
                    for hc in range(2):
                        nc.vector.tensor_mul(cA[:, hc, nsl], i_t[:, hc, :],
                                             u_t[:, hc, :])
                        nc.scalar.activation(out=u_t[:, hc, :],
                                             in_=cA[:, hc, nsl], func=AF.Tanh)
                        nc.vector.tensor_mul(hA[:, hc, nsl], o_t[:, hc, :],
                                             u_t[:, hc, :])
                    if ch == NCHUNK // 2 - 1:
                        for hc in range(2):
                            nc.sync.dma_start(out=out_d[hc, :, 0:LPC],
                                              in_=hA[:, hc, :])
                return s1, s2

            # ---- software-pipelined unit stream: leaves + internal levels ----
            units = [("leaf", ch, False) for ch in range(NCHUNK // 2)]
            cur = [hA, cA, hB, cB]
            lvl = 1
            n = LPC // 2
            while n >= CUT:
                h_src, c_src, h_dst, c_dst = cur
                # if the child level had <= 2 chunks, this level's first s1
                # reads h written by a pending s2 -> must flush the pipeline
                flush = (2 * n) // CH <= 2
                for ch in range(max(1, n // CH)):
                    units.append(("int", (n, ch, h_src, c_src, h_dst, c_dst,
                                          int(offs[lvl]), n == CUT),
                                  flush and ch == 0))
                cur = [cur[2], cur[3], cur[0], cur[1]]
                lvl += 1
                n //= 2
            pending = []
            for u in units:
                if u[2]:
                    while pending:
                        pending.pop(0)()
                s1, s2 = (unit_leaves(u[1]) if u[0] == "leaf"
                          else unit_internal(*u[1]))
                s1()
                pending.append(s2)
                if len(pending) > 1:
                    pending.pop(0)()
            for s2 in pending:
                s2()

    nc.compile()
    return nc, sizes, offs, TOT


class _ExecHandle:
    """Compiled SPMD executable with device-resident input support."""

    def __init__(self, nc):
        import jax
        from jax.sharding import Mesh, PartitionSpec
        try:
            from jax.experimental.shard_map import shard_map
        except ImportError:
            from jax.shard_map import shard_map
        from concourse import bass2jax, mybir

        bass2jax.install_neuronx_cc_hook()
        self.jax = jax
        partition_name = (nc.partition_id_tensor.name
                          if nc.partition_id_tensor else None)
        in_names, out_names, out_avals, zero_outs = [], [], [], []
        for alloc in nc.m.functions[0].allocations:
            if not isinstance(alloc, mybir.MemoryLocationSet):
                continue
            name = alloc.memorylocations[0].name
            if alloc.kind == "ExternalInput":
                if name != partition_name:
                    in_names.append(name)
            elif alloc.kind == "ExternalOutput":
                out_names.append(name)
                shape = tuple(alloc.tensor_shape)
                dtype = mybir.dt.np(alloc.dtype)
                out_avals.append(jax.core.ShapedArray(shape, dtype))
                zero_outs.append(np.zeros(shape, dtype))
        self.n_params = len(in_names)
        self.out_names = list(out_names)
        self.param_names = list(in_names)
        all_in_names = in_names + out_names
        if partition_name is not None:
            all_in_names.append(partition_name)
        self.out_avals = out_avals
        self.zero_outs = zero_outs

        def _body(*args):
            operands = list(args)
            if partition_name is not None:
                operands.append(bass2jax.partition_id_tensor())
            outs = bass2jax._bass_exec_p.bind(
                *operands,
                out_avals=tuple(out_avals),
                in_names=tuple(all_in_names),
                out_names=tuple(out_names),
                lowering_input_output_aliases=(),
                sim_require_finite=True,
                sim_require_nnan=True,
                nc=nc,
            )
            return tuple(outs)

        self._body = _body

        devices = jax.devices()[:NCORES]
        self.mesh = Mesh(np.asarray(devices), ("core",))
        n_ops = self.n_params + len(out_names)
        self.fn = jax.jit(shard_map(
            _body, mesh=self.mesh,
            in_specs=(PartitionSpec("core"),) * n_ops,
            out_specs=(PartitionSpec("core"),) * len(out_names),
            check_rep=False))

    def put_inputs(self, in_maps):
        import jax
        from jax.sharding import NamedSharding, PartitionSpec
        sh = NamedSharding(self.mesh, PartitionSpec("core"))
        ops = []
        for i, name in enumerate(self.param_names):
            arr = np.concatenate([np.asarray(m[name]) for m in in_maps], axis=0)
            ops.append(jax.device_put(arr, sh))
        for z in self.zero_outs:
            zz = np.zeros((NCORES * z.shape[0], *z.shape[1:]), z.dtype)
            ops.append(jax.device_put(zz, sh))
        return ops

    def run(self, ops):
        outs = self.fn(*ops)
        self.jax.block_until_ready(outs)
        return outs

    def results(self, outs):
        res = []
        for c in range(NCORES):
            d = {}
            for i, name in enumerate(self.out_names):
                a = np.asarray(outs[i])
                d[name] = a.reshape(NCORES, *self.out_avals[i].shape)[c]
            res.append(d)
        return res


def _sigmoid(z):
    with np.errstate(over="ignore"):
        return 1.0 / (1.0 + np.exp(-z))


_PROGRAM_CACHE = {}
_EXEC_CACHE = {}


def kernel(tokens, emb, Wx, Wl, Wr, b):
    global LAST_RESULTS, LAST_OPS
    tokens = np.asarray(tokens)
    emb = np.asarray(emb, dtype=np.float32)
    Wx = np.asarray(Wx, dtype=np.float32)
    Wl = np.asarray(Wl, dtype=np.float32)
    Wr = np.asarray(Wr, dtype=np.float32)
    b = np.asarray(b, dtype=np.float32)

    L = int(tokens.shape[0])
    LPC = L // NCORES
    mmdt = os.environ.get("TRNK_MM_DTYPE", "bfloat16")
    key = (LPC, mmdt)
    if key not in _PROGRAM_CACHE:
        _PROGRAM_CACHE[key] = _build_program(LPC, mmdt)
    nc, sizes, offs, TOT = _PROGRAM_CACHE[key]

    wt_blob = _pack_weights(Wx, Wl, Wr)
    bias_blob = np.ascontiguousarray(
        b.reshape(4, 2, 128).transpose(2, 0, 1).reshape(128, 8)).astype(np.float32)

    x = emb[tokens]  # [L, 256] host gather (input sharding/staging)
    rp = _revperm(LPC)
    if mmdt == "bfloat16":
        import ml_dtypes
        wt_blob = wt_blob.astype(ml_dtypes.bfloat16)
        cast = lambda a: a.astype(ml_dtypes.bfloat16)  # noqa: E731
    else:
        wt_blob = _round_fp32r(wt_blob)
        cast = _round_fp32r
    in_maps = []
    for ci in range(NCORES):
        xc = x[ci * LPC:(ci + 1) * LPC][rp]                   # stored order
        # [128, NCHUNK, 2, CH]: chunk-major so each chunk DMA is 2KB lines
        xblob = np.ascontiguousarray(
            xc.reshape(LPC // (2 * CH), 2 * CH, 2, 128).transpose(3, 0, 2, 1))
        in_maps.append({"x": cast(xblob), "wt": wt_blob, "bias": bias_blob})

    if key not in _EXEC_CACHE:
        _EXEC_CACHE[key] = _ExecHandle(nc)
    eh = _EXEC_CACHE[key]
    ops = eh.put_inputs(in_maps)
    outs = eh.run(ops)
    results = eh.results(outs)
    LAST_RESULTS = results
    LAST_OPS = ops

    # ---- host reassembly of device levels (global 32768 .. 2048) ----
    pieces = []
    for lvl, npc in enumerate(sizes):
        nglob = npc * NCORES
        rpl = _revperm(npc)
        lvlarr = np.empty((nglob, HIDDEN), np.float32)
        for ci in range(NCORES):
            o = results[ci]["out"]                      # [2, 128, TOT]
            st = o[:, :, offs[lvl]:offs[lvl] + npc].reshape(HIDDEN, npc)
            lvlarr[ci * npc:(ci + 1) * npc] = st.T[rpl].astype(np.float32)
        pieces.append(lvlarr)

    # ---- host tail: global levels 1024 .. 1 from per-core (h,c) at CUT ----
    rpc = _revperm(CUT)
    nglob = CUT * NCORES
    h = np.empty((nglob, HIDDEN), np.float32)
    c = np.empty((nglob, HIDDEN), np.float32)
    for ci in range(NCORES):
        st = results[ci]["out"][:, :, offs[-2]:offs[-2] + CUT]
        h[ci * CUT:(ci + 1) * CUT] = st.reshape(HIDDEN, CUT).T[rpc].astype(np.float32)
        stc = results[ci]["cend"]                       # [2, 128, CUT] f32
        c[ci * CUT:(ci + 1) * CUT] = stc.reshape(HIDDEN, CUT).T[rpc]

    # y = x @ W.T per gate; stack gates on columns: [in, 4*out]
    WlT = np.ascontiguousarray(Wl.transpose(2, 0, 1).reshape(HIDDEN, 4 * HIDDEN))
    WrT = np.ascontiguousarray(Wr.transpose(2, 0, 1).reshape(HIDDEN, 4 * HIDDEN))
    bfl = b.reshape(4 * HIDDEN)
    while h.shape[0] > 1:
        lh, rh = h[0::2], h[1::2]
        lc, rc = c[0::2], c[1::2]
        z = lh @ WlT + rh @ WrT + bfl                   # [n, 4H]
        i = _sigmoid(z[:, 0 * HIDDEN:1 * HIDDEN])
        f = _sigmoid(z[:, 1 * HIDDEN:2 * HIDDEN])
        o = _sigmoid(z[:, 2 * HIDDEN:3 * HIDDEN])
        u = np.tanh(z[:, 3 * HIDDEN:4 * HIDDEN])
        c = i * u + f * (lc + rc)
        h = o * np.tanh(c)
        pieces.append(h)
    return np.concatenate(pieces, axis=0)


# revision 62
# speedup vs baseline: 1.0001x; 1.0001x over previous
"""BinaryTreeLSTM on 8 Trainium2 NeuronCores (Bass/Tile).

Sharding: each core owns a contiguous subtree of 4096 leaves and reduces
it through the internal levels down to 1024 nodes per core (87.5% of
FLOPs, fully pipelined on device with no flushes). The remaining global
levels (8192 nodes down to the root) are latency-bound on device —
each costs a serial matmul+activation chain plus a pipeline flush for a
shrinking node count — so they are finished on the host with BLAS from
the per-core level-1024 h/c. This also removes the AllGather collective
entirely. Gate weights are replicated across cores.

Per-level node arrays are stored in bit-reversed node order on device so
each level's left/right children are the contiguous halves of the child
level; the host un-permutes when reassembling the natural level-order
output.

Device layout: hidden dim (256 = 2 chunks of 128) on the SBUF partition
axis, nodes on the free axis. Matmuls default to bf16 operands (fp32
PSUM accumulation; ~1e-3 output error) which halves input DMA and
enables fast weight loads; set TRNK_MM_DTYPE=float32r for the fp32r
path. Gate math is fp32 on the Scalar/Vector engines with the per-gate
bias folded into the activation instruction. Inputs stream in over both
hardware-DGE queues (Scalar + Sync) so descriptor generation for the
first-needed pieces overlaps.
"""

import os
import sys

import numpy as np

sys.path.insert(0, "/opt/trn_rl_repo")

HIDDEN = 256
NCORES = 8
CH = 512        # node-chunk (PSUM bank / fp32 moving-operand limit)
CUT = 1024      # smallest per-core level computed on device

# exposed for test harnesses
LAST_RESULTS = None
LAST_EXEC_NS = None
LAST_OPS = None


def _revperm(n):
    bits = n.bit_length() - 1
    r = np.arange(n)
    out = np.zeros(n, np.int64)
    for b in range(bits):
        out |= ((r >> b) & 1) << (bits - 1 - b)
    return out


def _w_tile_index(src, g, kc, hc):
    return ((src * 4 + g) * 2 + kc) * 2 + hc


def _round_fp32r(a):
    """Round fp32 values to the PE's fp32r format (1+8+11 bits, RNE)."""
    bits = np.ascontiguousarray(a, np.float32).view(np.uint32)
    odd = (bits >> np.uint32(12)) & np.uint32(1)
    bits = bits + np.uint32(0x7FF) + odd
    bits &= np.uint32(0xFFFFF000)
    return bits.view(np.float32)


def _pack_weights(Wx, Wl, Wr):
    # lhsT tile for (src, g, kc, hc): [p(contraction), m(out)] = W[g, hc*128+m, kc*128+p]
    tiles = []
    for W in (Wx, Wl, Wr):
        W4 = W.reshape(4, 2, 128, 2, 128)           # [g, hc, m, kc, p]
        tiles.append(W4.transpose(0, 3, 1, 4, 2))    # [g, kc, hc, p, m]
    allw = np.stack(tiles)                            # [3, 4, 2, 2, 128, 128]
    # -> [p, (s,g,kc,hc), m]
    blob = np.ascontiguousarray(allw.transpose(4, 0, 1, 2, 3, 5).reshape(128, 48, 128))
    return blob.astype(np.float32)


def _build_program(LPC, matmul_dtype_name="bfloat16"):
    from concourse import bacc, mybir, tile

    f32 = mybir.dt.float32
    mmdt = getattr(mybir.dt, matmul_dtype_name)
    bf16 = matmul_dtype_name == "bfloat16"
    sdt = mmdt if bf16 else f32
    AF = mybir.ActivationFunctionType

    sizes = []
    n = LPC
    while n >= CUT:
        sizes.append(n)
        n //= 2
    offs = np.concatenate([[0], np.cumsum(sizes)]).astype(int)
    TOT = int(offs[-1])
    NCHUNK = LPC // CH

    nc = bacc.Bacc("TRN2", target_bir_lowering=False, debug=False,
                   num_devices=NCORES)

    x_d = nc.dram_tensor("x", [128, NCHUNK // 2, 2, 2 * CH], mmdt,
                         kind="ExternalInput").ap()
    wt_d = nc.dram_tensor("wt", [128, 48, 128], mmdt, kind="ExternalInput").ap()
    bias_d = nc.dram_tensor("bias", [128, 8], f32, kind="ExternalInput").ap()
    out_d = nc.dram_tensor("out", [2, 128, TOT], sdt, kind="ExternalOutput").ap()
    cend_d = nc.dram_tensor("cend", [2, 128, CUT], f32, kind="ExternalOutput").ap()

    with tile.TileContext(nc) as tc:
        with tc.tile_pool(name="pp", bufs=1) as pp, \
             tc.tile_pool(name="zp", bufs=4, space="PSUM") as zp, \
             tc.tile_pool(name="gp", bufs=3) as gp:
            w_sb = pp.tile([128, 48, 128], mmdt, name="w_sb")
            bias_sb = pp.tile([128, 8], f32, name="bias_sb")
            hA = pp.tile([128, 2, LPC], sdt, name="hA")
            cA = pp.tile([128, 2, LPC], f32, name="cA")
            hB = pp.tile([128, 2, LPC // 2], sdt, name="hB")
            cB = pp.tile([128, 2, LPC // 2], f32, name="cB")
            x_sb = [pp.tile([128, 2, 2 * CH], mmdt, name=f"x_sb{ch}")
                    for ch in range(NCHUNK // 2)]

            # Inputs split across both HW-DGE queues (one queue per engine —
            # same-engine DMAs serialize): bias + leaf weights on Scalar,
            # x chunks on Sync, so the first matmul's inputs land early.
            nc.scalar.dma_start(out=bias_sb[:], in_=bias_d[:])
            nc.scalar.dma_start(out=w_sb[:, 0:16, :], in_=wt_d[:, 0:16, :])
            nc.scalar.dma_start(out=x_sb[1][:], in_=x_d[:, 1])
            nc.scalar.dma_start(out=w_sb[:, 16:48, :], in_=wt_d[:, 16:48, :])
            for ch in (0, 2, 3):
                nc.sync.dma_start(out=x_sb[ch][:], in_=x_d[:, ch])

            if bf16:
                cast_rhs = lambda ap: ap  # noqa: E731
            else:
                cast_rhs = lambda ap: ap.bitcast(mmdt)  # noqa: E731

            def mm(w_idx, rhs_ap, zt, start, stop):
                nc.tensor.matmul(zt, w_sb[:, w_idx, :], cast_rhs(rhs_ap),
                                 start=start, stop=stop)

            def unit_internal(n, ch, h_src, c_src, h_dst, c_dst, out_off,
                              c_out=False):
                """One chunk of an internal level -> (stage1, stage2)."""
                nchunks = max(1, n // CH)
                m = min(n, CH)
                lsl = slice(ch * m, (ch + 1) * m)
                rsl = slice(n + ch * m, n + (ch + 1) * m)
                dsl = slice(ch * m, (ch + 1) * m)
                i_t = gp.tile([128, 2, CH], f32, name="i_t")
                f_t = gp.tile([128, 2, CH], f32, name="f_t")
                o_t = gp.tile([128, 2, CH], f32, name="o_t")
                u_t = gp.tile([128, 2, CH], f32, name="u_t")
                s_t = gp.tile([128, 2, CH], f32, name="s_t")
                gates = {0: i_t, 1: f_t, 2: o_t, 3: u_t}

                def s1():
                    for hc in range(2):
                        nc.gpsimd.tensor_add(s_t[:, hc, :m],
                                             c_src[:, hc, lsl],
                                             c_src[:, hc, rsl])
                    for hc in range(2):
                        for g in (0, 3, 1, 2):
                            zt = zp.tile([128, 2 * CH], f32, name="zt")
                            mm(_w_tile_index(1, g, 0, hc), h_src[:, 0, lsl],
                               zt[:, :m], True, False)
                            mm(_w_tile_index(1, g, 1, hc), h_src[:, 1, lsl],
                               zt[:, :m], False, False)
                            mm(_w_tile_index(2, g, 0, hc), h_src[:, 0, rsl],
                               zt[:, :m], False, False)
                            mm(_w_tile_index(2, g, 1, hc), h_src[:, 1, rsl],
                               zt[:, :m], False, True)
                            func = AF.Tanh if g == 3 else AF.Sigmoid
                            nc.scalar.activation(
                                out=gates[g][:, hc, :m], in_=zt[:, :m],
                                func=func,
                                bias=bias_sb[:, g * 2 + hc:g * 2 + hc + 1])

                def s2():
                    for hc in range(2):
                        nc.vector.tensor_mul(u_t[:, hc, :m], i_t[:, hc, :m],
                                             u_t[:, hc, :m])
                        nc.vector.tensor_mul(s_t[:, hc, :m], f_t[:, hc, :m],
                                             s_t[:, hc, :m])
                        nc.vector.tensor_add(c_dst[:, hc, dsl],
                                             u_t[:, hc, :m], s_t[:, hc, :m])
                        nc.scalar.activation(out=i_t[:, hc, :m],
                                             in_=c_dst[:, hc, dsl],
                                             func=AF.Tanh)
                        nc.vector.tensor_mul(h_dst[:, hc, dsl],
                                             o_t[:, hc, :m], i_t[:, hc, :m])
                    if ch == nchunks - 1:
                        for hc in range(2):
                            nc.sync.dma_start(
                                out=out_d[hc, :, out_off:out_off + n],
                                in_=h_dst[:, hc, :n])
                        if c_out:
                            for hc in range(2):
                                nc.sync.dma_start(
                                    out=cend_d[hc, :, :],
                                    in_=c_dst[:, hc, :n])
                return s1, s2

            def unit_leaves(ch):
                # 1024-node super-chunk: z spans 2 PSUM banks (each matmul
                # stays within one bank) so a single 1024-column activation
                # with one bias covers the whole chunk per (g, hc)
                LCH = 2 * CH
                nsl = slice(ch * LCH, (ch + 1) * LCH)
                xc_t = x_sb[ch]
                i_t = gp.tile([128, 2, LCH], sdt, name="li_t")
                o_t = gp.tile([128, 2, LCH], sdt, name="lo_t")
                u_t = gp.tile([128, 2, LCH], sdt, name="lu_t")
                lgates = {0: i_t, 2: o_t, 3: u_t}

                def s1():
                    for hc in range(2):
                        for g in (0, 3, 2):
                            zt = zp.tile([128, LCH], f32, name="zt")
                            for half in (0, CH):
                                hsl = slice(half, half + CH)
                                mm(_w_tile_index(0, g, 0, hc),
                                   xc_t[:, 0, hsl], zt[:, hsl], True, False)
                                mm(_w_tile_index(0, g, 1, hc),
                                   xc_t[:, 1, hsl], zt[:, hsl], False, True)
                            func = AF.Tanh if g == 3 else AF.Sigmoid
                            nc.scalar.activation(
                                out=lgates[g][:, hc, :], in_=zt[:], func=func,
                                bias=bias_sb[:, g * 2 + hc:g * 2 + hc + 1])

                def s2():
                    for hc in range(2):
                        nc.vector.tensor_mul(cA[:, hc, nsl], i_t[:, hc, :],
                                             u_t[:, hc, :])
                        nc.scalar.activation(out=u_t[:, hc, :],
                                             in_=cA[:, hc, nsl], func=AF.Tanh)
                        nc.vector.tensor_mul(hA[:, hc, nsl], o_t[:, hc, :],
                                             u_t[:, hc, :])
                    if ch == NCHUNK // 2 - 1:
                        for hc in range(2):
                            nc.sync.dma_start(out=out_d[hc, :, 0:LPC],
                                              in_=hA[:, hc, :])
                return s1, s2

            # ---- software-pipelined unit stream: leaves + internal levels ----
            units = [("leaf", ch, False) for ch in range(NCHUNK // 2)]
            cur = [hA, cA, hB, cB]
            lvl = 1
            n = LPC // 2
            while n >= CUT:
                h_src, c_src, h_dst, c_dst = cur
                # if the child level had <= 2 chunks, this level's first s1
                # reads h written by a pending s2 -> must flush the pipeline
                flush = (2 * n) // CH <= 2
                for ch in range(max(1, n // CH)):
                    units.append(("int", (n, ch, h_src, c_src, h_dst, c_dst,
                                          int(offs[lvl]), n == CUT),
                                  flush and ch == 0))
                cur = [cur[2], cur[3], cur[0], cur[1]]
                lvl += 1
                n //= 2
            pending = []
            for u in units:
                if u[2]:
                    while pending:
                        pending.pop(0)()
                s1, s2 = (unit_leaves(u[1]) if u[0] == "leaf"
                          else unit_internal(*u[1]))
                s1()
                pending.append(s2)
                if len(pending) > 1:
                    pending.pop(0)()
            for s2 in pending:
                s2()

    nc.compile()
    return nc, sizes, offs, TOT


class _ExecHandle:
    """Compiled SPMD executable with device-resident input support."""

    def __init__(self, nc):
        import jax
        from jax.sharding import Mesh, PartitionSpec
        try:
            from jax.experimental.shard_map import shard_map
        except ImportError:
            from jax.shard_map import shard_map
        from concourse import bass2jax, mybir

        bass2jax.install_neuronx_cc_hook()
        self.jax = jax
        partition_name = (nc.partition_id_tensor.name
                          if nc.partition_id_tensor else None)
        in_names, out_names, out_avals, zero_outs = [], [], [], []
        for alloc in nc.m.functions[0].allocations:
            if not isinstance(alloc, mybir.MemoryLocationSet):
                continue
            name = alloc.memorylocations[0].name
            if alloc.kind == "ExternalInput":
                if name != partition_name:
                    in_names.append(name)
            elif alloc.kind == "ExternalOutput":
                out_names.append(name)
                shape = tuple(alloc.tensor_shape)
                dtype = mybir.dt.np(alloc.dtype)
                out_avals.append(jax.core.ShapedArray(shape, dtype))
                zero_outs.append(np.zeros(shape, dtype))
        self.n_params = len(in_names)
        self.out_names = list(out_names)
        self.param_names = list(in_names)
        all_in_names = in_names + out_names
        if partition_name is not None:
            all_in_names.append(partition_name)
        self.out_avals = out_avals
        self.zero_outs = zero_outs

        def _body(*args):
            operands = list(args)
            if partition_name is not None:
                operands.append(bass2jax.partition_id_tensor())
            outs = bass2jax._bass_exec_p.bind(
                *operands,
                out_avals=tuple(out_avals),
                in_names=tuple(all_in_names),
                out_names=tuple(out_names),
                lowering_input_output_aliases=(),
                sim_require_finite=True,
                sim_require_nnan=True,
                nc=nc,
            )
            return tuple(outs)

        self._body = _body

        devices = jax.devices()[:NCORES]
        self.mesh = Mesh(np.asarray(devices), ("core",))
        n_ops = self.n_params + len(out_names)
        self.fn = jax.jit(shard_map(
            _body, mesh=self.mesh,
            in_specs=(PartitionSpec("core"),) * n_ops,
            out_specs=(PartitionSpec("core"),) * len(out_names),
            check_rep=False))

    def put_inputs(self, in_maps):
        import jax
        from jax.sharding import NamedSharding, PartitionSpec
        sh = NamedSharding(self.mesh, PartitionSpec("core"))
        ops = []
        for i, name in enumerate(self.param_names):
            arr = np.concatenate([np.asarray(m[name]) for m in in_maps], axis=0)
            ops.append(jax.device_put(arr, sh))
        for z in self.zero_outs:
            zz = np.zeros((NCORES * z.shape[0], *z.shape[1:]), z.dtype)
            ops.append(jax.device_put(zz, sh))
        return ops

    def run(self, ops):
        outs = self.fn(*ops)
        self.jax.block_until_ready(outs)
        return outs

    def results(self, outs):
        res = []
        for c in range(NCORES):
            d = {}
            for i, name in enumerate(self.out_names):
                a = np.asarray(outs[i])
                d[name] = a.reshape(NCORES, *self.out_avals[i].shape)[c]
            res.append(d)
        return res


def _sigmoid(z):
    with np.errstate(over="ignore"):
        return 1.0 / (1.0 + np.exp(-z))


_PROGRAM_CACHE = {}
_EXEC_CACHE = {}


def kernel(tokens, emb, Wx, Wl, Wr, b):
    global LAST_RESULTS, LAST_OPS
    tokens = np.asarray(tokens)
    emb = np.asarray(emb, dtype=np.float32)
    Wx = np.asarray(Wx, dtype=np.float32)
    Wl = np.asarray(Wl, dtype=np.float32)
    Wr = np.asarray(Wr, dtype=np.float32)
    b = np.asarray(b, dtype=np.float32)

    L = int(tokens.shape[0])
    LPC = L // NCORES
    mmdt = os.environ.get("TRNK_MM_DTYPE", "bfloat16")
    key = (LPC, mmdt)
    if key not in _PROGRAM_CACHE:
        _PROGRAM_CACHE[key] = _build_program(LPC, mmdt)
    nc, sizes, offs, TOT = _PROGRAM_CACHE[key]

    wt_blob = _pack_weights(Wx, Wl, Wr)
    bias_blob = np.ascontiguousarray(
        b.reshape(4, 2, 128).transpose(2, 0, 1).reshape(128, 8)).astype(np.float32)

    x = emb[tokens]  # [L, 256] host gather (input sharding/staging)
    rp = _revperm(LPC)
    if mmdt == "bfloat16":
        import ml_dtypes
        wt_blob = wt_blob.astype(ml_dtypes.bfloat16)
        cast = lambda a: a.astype(ml_dtypes.bfloat16)  # noqa: E731
    else:
        wt_blob = _round_fp32r(wt_blob)
        cast = _round_fp32r
    in_maps = []
    for ci in range(NCORES):
        xc = x[ci * LPC:(ci + 1) * LPC][rp]                   # stored order
        # [128, NCHUNK, 2, CH]: chunk-major so each chunk DMA is 2KB lines
        xblob = np.ascontiguousarray(
            xc.reshape(LPC // (2 * CH), 2 * CH, 2, 128).transpose(3, 0, 2, 1))
        in_maps.append({"x": cast(xblob), "wt": wt_blob, "bias": bias_blob})

    if key not in _EXEC_CACHE:
        _EXEC_CACHE[key] = _ExecHandle(nc)
    eh = _EXEC_CACHE[key]
    ops = eh.put_inputs(in_maps)
    outs = eh.run(ops)
    results = eh.results(outs)
    LAST_RESULTS = results
    LAST_OPS = ops

    # ---- host reassembly of device levels (global 32768 .. 2048) ----
    pieces = []
    for lvl, npc in enumerate(sizes):
        nglob = npc * NCORES
        rpl = _revperm(npc)
        lvlarr = np.empty((nglob, HIDDEN), np.float32)
        for ci in range(NCORES):
            o = results[ci]["out"]                      # [2, 128, TOT]
            st = o[:, :, offs[lvl]:offs[lvl] + npc].reshape(HIDDEN, npc)
            lvlarr[ci * npc:(ci + 1) * npc] = st.T[rpl].astype(np.float32)
        pieces.append(lvlarr)

    # ---- host tail: global levels 1024 .. 1 from per-core (h,c) at CUT ----
    rpc = _revperm(CUT)
    nglob = CUT * NCORES
    h = np.empty((nglob, HIDDEN), np.float32)
    c = np.empty((nglob, HIDDEN), np.float32)
    for ci in range(NCORES):
        st = results[ci]["out"][:, :, offs[-2]:offs[-2] + CUT]
        h[ci * CUT:(ci + 1) * CUT] = st.reshape(HIDDEN, CUT).T[rpc].astype(np.float32)
        stc = results[ci]["cend"]                       # [2, 128, CUT] f32
        c[ci * CUT:(ci + 1) * CUT] = stc.reshape(HIDDEN, CUT).T[rpc]

    # y = x @ W.T per gate; stack gates on columns: [in, 4*out]
    WlT = np.ascontiguousarray(Wl.transpose(2, 0, 1).reshape(HIDDEN, 4 * HIDDEN))
    WrT = np.ascontiguousarray(Wr.transpose(2, 0, 1).reshape(HIDDEN, 4 * HIDDEN))
    bfl = b.reshape(4 * HIDDEN)
    while h.shape[0] > 1:
        lh, rh = h[0::2], h[1::2]
        lc, rc = c[0::2], c[1::2]
        z = lh @ WlT + rh @ WrT + bfl                   # [n, 4H]
        i = _sigmoid(z[:, 0 * HIDDEN:1 * HIDDEN])
        f = _sigmoid(z[:, 1 * HIDDEN:2 * HIDDEN])
        o = _sigmoid(z[:, 2 * HIDDEN:3 * HIDDEN])
        u = np.tanh(z[:, 3 * HIDDEN:4 * HIDDEN])
        c = i * u + f * (lc + rc)
        h = o * np.tanh(c)
        pieces.append(h)
    return np.concatenate(pieces, axis=0)


# revision 63
# speedup vs baseline: 1.0176x; 1.0175x over previous
"""BinaryTreeLSTM on 8 Trainium2 NeuronCores (Bass/Tile).

Sharding: each core owns a contiguous subtree of 4096 leaves and reduces
it through the internal levels down to 1024 nodes per core (87.5% of
FLOPs, fully pipelined on device with no flushes). The remaining global
levels (8192 nodes down to the root) are latency-bound on device —
each costs a serial matmul+activation chain plus a pipeline flush for a
shrinking node count — so they are finished on the host with BLAS from
the per-core level-1024 h/c. This also removes the AllGather collective
entirely. Gate weights are replicated across cores.

Per-level node arrays are stored in bit-reversed node order on device so
each level's left/right children are the contiguous halves of the child
level; the host un-permutes when reassembling the natural level-order
output.

Device layout: hidden dim (256 = 2 chunks of 128) on the SBUF partition
axis, nodes on the free axis. Matmuls default to bf16 operands (fp32
PSUM accumulation; ~1e-3 output error) which halves input DMA and
enables fast weight loads; set TRNK_MM_DTYPE=float32r for the fp32r
path. Gate math is fp32 on the Scalar/Vector engines with the per-gate
bias folded into the activation instruction. Inputs stream in over both
hardware-DGE queues (Scalar + Sync) so descriptor generation for the
first-needed pieces overlaps.
"""

import os
import sys

import numpy as np

sys.path.insert(0, "/opt/trn_rl_repo")

HIDDEN = 256
NCORES = 8
CH = 512        # node-chunk (PSUM bank / fp32 moving-operand limit)
CUT = 1024      # smallest per-core level computed on device

# exposed for test harnesses
LAST_RESULTS = None
LAST_EXEC_NS = None
LAST_OPS = None


def _revperm(n):
    bits = n.bit_length() - 1
    r = np.arange(n)
    out = np.zeros(n, np.int64)
    for b in range(bits):
        out |= ((r >> b) & 1) << (bits - 1 - b)
    return out


def _w_tile_index(src, g, kc, hc):
    return ((src * 4 + g) * 2 + kc) * 2 + hc


def _round_fp32r(a):
    """Round fp32 values to the PE's fp32r format (1+8+11 bits, RNE)."""
    bits = np.ascontiguousarray(a, np.float32).view(np.uint32)
    odd = (bits >> np.uint32(12)) & np.uint32(1)
    bits = bits + np.uint32(0x7FF) + odd
    bits &= np.uint32(0xFFFFF000)
    return bits.view(np.float32)


def _pack_weights(Wx, Wl, Wr):
    # lhsT tile for (src, g, kc, hc): [p(contraction), m(out)] = W[g, hc*128+m, kc*128+p]
    tiles = []
    for W in (Wx, Wl, Wr):
        W4 = W.reshape(4, 2, 128, 2, 128)           # [g, hc, m, kc, p]
        tiles.append(W4.transpose(0, 3, 1, 4, 2))    # [g, kc, hc, p, m]
    allw = np.stack(tiles)                            # [3, 4, 2, 2, 128, 128]
    # -> [p, (s,g,kc,hc), m]
    blob = np.ascontiguousarray(allw.transpose(4, 0, 1, 2, 3, 5).reshape(128, 48, 128))
    return blob.astype(np.float32)


def _build_program(LPC, matmul_dtype_name="bfloat16"):
    from concourse import bacc, mybir, tile

    f32 = mybir.dt.float32
    mmdt = getattr(mybir.dt, matmul_dtype_name)
    bf16 = matmul_dtype_name == "bfloat16"
    sdt = mmdt if bf16 else f32
    AF = mybir.ActivationFunctionType

    sizes = []
    n = LPC
    while n >= CUT:
        sizes.append(n)
        n //= 2
    offs = np.concatenate([[0], np.cumsum(sizes)]).astype(int)
    TOT = int(offs[-1])
    NCHUNK = LPC // CH

    nc = bacc.Bacc("TRN2", target_bir_lowering=False, debug=False,
                   num_devices=NCORES)

    x_d = nc.dram_tensor("x", [128, NCHUNK, 2, CH], mmdt,
                         kind="ExternalInput").ap()
    wt_d = nc.dram_tensor("wt", [128, 48, 128], mmdt, kind="ExternalInput").ap()
    bias_d = nc.dram_tensor("bias", [128, 8], f32, kind="ExternalInput").ap()
    out_d = nc.dram_tensor("out", [2, 128, TOT], sdt, kind="ExternalOutput").ap()
    cend_d = nc.dram_tensor("cend", [2, 128, CUT], f32, kind="ExternalOutput").ap()

    with tile.TileContext(nc) as tc:
        with tc.tile_pool(name="pp", bufs=1) as pp, \
             tc.tile_pool(name="zp", bufs=6, space="PSUM") as zp, \
             tc.tile_pool(name="gp", bufs=3) as gp:
            w_sb = pp.tile([128, 48, 128], mmdt, name="w_sb")
            bias_sb = pp.tile([128, 8], f32, name="bias_sb")
            hA = pp.tile([128, 2, LPC], sdt, name="hA")
            cA = pp.tile([128, 2, LPC], f32, name="cA")
            hB = pp.tile([128, 2, LPC // 2], sdt, name="hB")
            cB = pp.tile([128, 2, LPC // 2], f32, name="cB")
            x_sb = [pp.tile([128, 2, CH], mmdt, name=f"x_sb{ch}")
                    for ch in range(NCHUNK)]

            # Inputs split across both HW-DGE queues (one queue per engine —
            # same-engine DMAs serialize): bias + leaf weights on Scalar,
            # x chunks on Sync, so the first matmul's inputs land early.
            nc.scalar.dma_start(out=bias_sb[:], in_=bias_d[:])
            nc.scalar.dma_start(out=w_sb[:, 0:16, :], in_=wt_d[:, 0:16, :])
            for ch in (2, 5):
                nc.scalar.dma_start(out=x_sb[ch][:], in_=x_d[:, ch])
            nc.scalar.dma_start(out=w_sb[:, 16:48, :], in_=wt_d[:, 16:48, :])
            for ch in (0, 1, 3, 4, 6, 7):
                nc.sync.dma_start(out=x_sb[ch][:], in_=x_d[:, ch])

            if bf16:
                cast_rhs = lambda ap: ap  # noqa: E731
            else:
                cast_rhs = lambda ap: ap.bitcast(mmdt)  # noqa: E731

            def mm(w_idx, rhs_ap, zt, start, stop):
                nc.tensor.matmul(zt, w_sb[:, w_idx, :], cast_rhs(rhs_ap),
                                 start=start, stop=stop)

            def unit_internal(n, ch, h_src, c_src, h_dst, c_dst, out_off,
                              c_out=False):
                """One chunk of an internal level -> (stage1, stage2)."""
                nchunks = max(1, n // CH)
                m = min(n, CH)
                lsl = slice(ch * m, (ch + 1) * m)
                rsl = slice(n + ch * m, n + (ch + 1) * m)
                dsl = slice(ch * m, (ch + 1) * m)
                i_t = gp.tile([128, 2, CH], f32, name="i_t")
                f_t = gp.tile([128, 2, CH], f32, name="f_t")
                o_t = gp.tile([128, 2, CH], f32, name="o_t")
                u_t = gp.tile([128, 2, CH], f32, name="u_t")
                s_t = gp.tile([128, 2, CH], f32, name="s_t")
                gates = {0: i_t, 1: f_t, 2: o_t, 3: u_t}

                def s1():
                    for hc in range(2):
                        nc.gpsimd.tensor_add(s_t[:, hc, :m],
                                             c_src[:, hc, lsl],
                                             c_src[:, hc, rsl])
                    for hc in range(2):
                        for g in (0, 3, 1, 2):
                            zt = zp.tile([128, CH], f32, name="zt")
                            mm(_w_tile_index(1, g, 0, hc), h_src[:, 0, lsl],
                               zt[:, :m], True, False)
                            mm(_w_tile_index(1, g, 1, hc), h_src[:, 1, lsl],
                               zt[:, :m], False, False)
                            mm(_w_tile_index(2, g, 0, hc), h_src[:, 0, rsl],
                               zt[:, :m], False, False)
                            mm(_w_tile_index(2, g, 1, hc), h_src[:, 1, rsl],
                               zt[:, :m], False, True)
                            func = AF.Tanh if g == 3 else AF.Sigmoid
                            nc.scalar.activation(
                                out=gates[g][:, hc, :m], in_=zt[:, :m],
                                func=func,
                                bias=bias_sb[:, g * 2 + hc:g * 2 + hc + 1])

                def s2():
                    for hc in range(2):
                        nc.vector.tensor_mul(u_t[:, hc, :m], i_t[:, hc, :m],
                                             u_t[:, hc, :m])
                        nc.vector.tensor_mul(s_t[:, hc, :m], f_t[:, hc, :m],
                                             s_t[:, hc, :m])
                        nc.vector.tensor_add(c_dst[:, hc, dsl],
                                             u_t[:, hc, :m], s_t[:, hc, :m])
                        nc.scalar.activation(out=i_t[:, hc, :m],
                                             in_=c_dst[:, hc, dsl],
                                             func=AF.Tanh)
                        nc.vector.tensor_mul(h_dst[:, hc, dsl],
                                             o_t[:, hc, :m], i_t[:, hc, :m])
                    if ch == nchunks - 1:
                        for hc in range(2):
                            nc.sync.dma_start(
                                out=out_d[hc, :, out_off:out_off + n],
                                in_=h_dst[:, hc, :n])
                        if c_out:
                            for hc in range(2):
                                nc.sync.dma_start(
                                    out=cend_d[hc, :, :],
                                    in_=c_dst[:, hc, :n])
                return s1, s2

            def unit_leaves(ch):
                nsl = slice(ch * CH, (ch + 1) * CH)
                xc_t = x_sb[ch]
                i_t = gp.tile([128, 2, CH], f32, name="i_t")
                o_t = gp.tile([128, 2, CH], f32, name="o_t")
                u_t = gp.tile([128, 2, CH], f32, name="u_t")
                lgates = {0: i_t, 2: o_t, 3: u_t}

                def s1():
                    for hc in range(2):
                        for g in (0, 3, 2):
                            zt = zp.tile([128, CH], f32, name="zt")
                            mm(_w_tile_index(0, g, 0, hc), xc_t[:, 0, :],
                               zt[:], True, False)
                            mm(_w_tile_index(0, g, 1, hc), xc_t[:, 1, :],
                               zt[:], False, True)
                            func = AF.Tanh if g == 3 else AF.Sigmoid
                            nc.scalar.activation(
                                out=lgates[g][:, hc, :], in_=zt[:], func=func,
                                bias=bias_sb[:, g * 2 + hc:g * 2 + hc + 1])

                def s2():
                    for hc in range(2):
                        nc.vector.tensor_mul(cA[:, hc, nsl], i_t[:, hc, :],
                                             u_t[:, hc, :])
                        nc.scalar.activation(out=u_t[:, hc, :],
                                             in_=cA[:, hc, nsl], func=AF.Tanh)
                        nc.vector.tensor_mul(hA[:, hc, nsl], o_t[:, hc, :],
                                             u_t[:, hc, :])
                    if ch == NCHUNK - 1:
                        for hc in range(2):
                            nc.sync.dma_start(out=out_d[hc, :, 0:LPC],
                                              in_=hA[:, hc, :])
                return s1, s2

            # ---- software-pipelined unit stream: leaves + internal levels ----
            units = [("leaf", ch, False) for ch in range(NCHUNK)]
            cur = [hA, cA, hB, cB]
            lvl = 1
            n = LPC // 2
            while n >= CUT:
                h_src, c_src, h_dst, c_dst = cur
                # if the child level had <= 2 chunks, this level's first s1
                # reads h written by a pending s2 -> must flush the pipeline
                flush = (2 * n) // CH <= 2
                for ch in range(max(1, n // CH)):
                    units.append(("int", (n, ch, h_src, c_src, h_dst, c_dst,
                                          int(offs[lvl]), n == CUT),
                                  flush and ch == 0))
                cur = [cur[2], cur[3], cur[0], cur[1]]
                lvl += 1
                n //= 2
            pending = []
            for u in units:
                if u[2]:
                    while pending:
                        pending.pop(0)()
                s1, s2 = (unit_leaves(u[1]) if u[0] == "leaf"
                          else unit_internal(*u[1]))
                s1()
                pending.append(s2)
                if len(pending) > 1:
                    pending.pop(0)()
            for s2 in pending:
                s2()

    nc.compile()
    return nc, sizes, offs, TOT


class _ExecHandle:
    """Compiled SPMD executable with device-resident input support."""

    def __init__(self, nc):
        import jax
        from jax.sharding import Mesh, PartitionSpec
        try:
            from jax.experimental.shard_map import shard_map
        except ImportError:
            from jax.shard_map import shard_map
        from concourse import bass2jax, mybir

        bass2jax.install_neuronx_cc_hook()
        self.jax = jax
        partition_name = (nc.partition_id_tensor.name
                          if nc.partition_id_tensor else None)
        in_names, out_names, out_avals, zero_outs = [], [], [], []
        for alloc in nc.m.functions[0].allocations:
            if not isinstance(alloc, mybir.MemoryLocationSet):
                continue
            name = alloc.memorylocations[0].name
            if alloc.kind == "ExternalInput":
                if name != partition_name:
                    in_names.append(name)
            elif alloc.kind == "ExternalOutput":
                out_names.append(name)
                shape = tuple(alloc.tensor_shape)
                dtype = mybir.dt.np(alloc.dtype)
                out_avals.append(jax.core.ShapedArray(shape, dtype))
                zero_outs.append(np.zeros(shape, dtype))
        self.n_params = len(in_names)
        self.out_names = list(out_names)
        self.param_names = list(in_names)
        all_in_names = in_names + out_names
        if partition_name is not None:
            all_in_names.append(partition_name)
        self.out_avals = out_avals
        self.zero_outs = zero_outs

        def _body(*args):
            operands = list(args)
            if partition_name is not None:
                operands.append(bass2jax.partition_id_tensor())
            outs = bass2jax._bass_exec_p.bind(
                *operands,
                out_avals=tuple(out_avals),
                in_names=tuple(all_in_names),
                out_names=tuple(out_names),
                lowering_input_output_aliases=(),
                sim_require_finite=True,
                sim_require_nnan=True,
                nc=nc,
            )
            return tuple(outs)

        self._body = _body

        devices = jax.devices()[:NCORES]
        self.mesh = Mesh(np.asarray(devices), ("core",))
        n_ops = self.n_params + len(out_names)
        self.fn = jax.jit(shard_map(
            _body, mesh=self.mesh,
            in_specs=(PartitionSpec("core"),) * n_ops,
            out_specs=(PartitionSpec("core"),) * len(out_names),
            check_rep=False))

    def put_inputs(self, in_maps):
        import jax
        from jax.sharding import NamedSharding, PartitionSpec
        sh = NamedSharding(self.mesh, PartitionSpec("core"))
        ops = []
        for i, name in enumerate(self.param_names):
            arr = np.concatenate([np.asarray(m[name]) for m in in_maps], axis=0)
            ops.append(jax.device_put(arr, sh))
        for z in self.zero_outs:
            zz = np.zeros((NCORES * z.shape[0], *z.shape[1:]), z.dtype)
            ops.append(jax.device_put(zz, sh))
        return ops

    def run(self, ops):
        outs = self.fn(*ops)
        self.jax.block_until_ready(outs)
        return outs

    def results(self, outs):
        res = []
        for c in range(NCORES):
            d = {}
            for i, name in enumerate(self.out_names):
                a = np.asarray(outs[i])
                d[name] = a.reshape(NCORES, *self.out_avals[i].shape)[c]
            res.append(d)
        return res


def _sigmoid(z):
    with np.errstate(over="ignore"):
        return 1.0 / (1.0 + np.exp(-z))


_PROGRAM_CACHE = {}
_EXEC_CACHE = {}


def kernel(tokens, emb, Wx, Wl, Wr, b):
    global LAST_RESULTS, LAST_OPS
    tokens = np.asarray(tokens)
    emb = np.asarray(emb, dtype=np.float32)
    Wx = np.asarray(Wx, dtype=np.float32)
    Wl = np.asarray(Wl, dtype=np.float32)
    Wr = np.asarray(Wr, dtype=np.float32)
    b = np.asarray(b, dtype=np.float32)

    L = int(tokens.shape[0])
    LPC = L // NCORES
    mmdt = os.environ.get("TRNK_MM_DTYPE", "bfloat16")
    key = (LPC, mmdt)
    if key not in _PROGRAM_CACHE:
        _PROGRAM_CACHE[key] = _build_program(LPC, mmdt)
    nc, sizes, offs, TOT = _PROGRAM_CACHE[key]

    wt_blob = _pack_weights(Wx, Wl, Wr)
    bias_blob = np.ascontiguousarray(
        b.reshape(4, 2, 128).transpose(2, 0, 1).reshape(128, 8)).astype(np.float32)

    x = emb[tokens]  # [L, 256] host gather (input sharding/staging)
    rp = _revperm(LPC)
    if mmdt == "bfloat16":
        import ml_dtypes
        wt_blob = wt_blob.astype(ml_dtypes.bfloat16)
        cast = lambda a: a.astype(ml_dtypes.bfloat16)  # noqa: E731
    else:
        wt_blob = _round_fp32r(wt_blob)
        cast = _round_fp32r
    in_maps = []
    for ci in range(NCORES):
        xc = x[ci * LPC:(ci + 1) * LPC][rp]                   # stored order
        # [128, NCHUNK, 2, CH]: chunk-major so each chunk DMA is 2KB lines
        xblob = np.ascontiguousarray(
            xc.reshape(LPC // CH, CH, 2, 128).transpose(3, 0, 2, 1))
        in_maps.append({"x": cast(xblob), "wt": wt_blob, "bias": bias_blob})

    if key not in _EXEC_CACHE:
        _EXEC_CACHE[key] = _ExecHandle(nc)
    eh = _EXEC_CACHE[key]
    ops = eh.put_inputs(in_maps)
    outs = eh.run(ops)
    results = eh.results(outs)
    LAST_RESULTS = results
    LAST_OPS = ops

    # ---- host reassembly of device levels (global 32768 .. 2048) ----
    pieces = []
    for lvl, npc in enumerate(sizes):
        nglob = npc * NCORES
        rpl = _revperm(npc)
        lvlarr = np.empty((nglob, HIDDEN), np.float32)
        for ci in range(NCORES):
            o = results[ci]["out"]                      # [2, 128, TOT]
            st = o[:, :, offs[lvl]:offs[lvl] + npc].reshape(HIDDEN, npc)
            lvlarr[ci * npc:(ci + 1) * npc] = st.T[rpl].astype(np.float32)
        pieces.append(lvlarr)

    # ---- host tail: global levels 1024 .. 1 from per-core (h,c) at CUT ----
    rpc = _revperm(CUT)
    nglob = CUT * NCORES
    h = np.empty((nglob, HIDDEN), np.float32)
    c = np.empty((nglob, HIDDEN), np.float32)
    for ci in range(NCORES):
        st = results[ci]["out"][:, :, offs[-2]:offs[-2] + CUT]
        h[ci * CUT:(ci + 1) * CUT] = st.reshape(HIDDEN, CUT).T[rpc].astype(np.float32)
        stc = results[ci]["cend"]                       # [2, 128, CUT] f32
        c[ci * CUT:(ci + 1) * CUT] = stc.reshape(HIDDEN, CUT).T[rpc]

    # y = x @ W.T per gate; stack gates on columns: [in, 4*out]
    WlT = np.ascontiguousarray(Wl.transpose(2, 0, 1).reshape(HIDDEN, 4 * HIDDEN))
    WrT = np.ascontiguousarray(Wr.transpose(2, 0, 1).reshape(HIDDEN, 4 * HIDDEN))
    bfl = b.reshape(4 * HIDDEN)
    while h.shape[0] > 1:
        lh, rh = h[0::2], h[1::2]
        lc, rc = c[0::2], c[1::2]
        z = lh @ WlT + rh @ WrT + bfl                   # [n, 4H]
        i = _sigmoid(z[:, 0 * HIDDEN:1 * HIDDEN])
        f = _sigmoid(z[:, 1 * HIDDEN:2 * HIDDEN])
        o = _sigmoid(z[:, 2 * HIDDEN:3 * HIDDEN])
        u = np.tanh(z[:, 3 * HIDDEN:4 * HIDDEN])
        c = i * u + f * (lc + rc)
        h = o * np.tanh(c)
        pieces.append(h)
    return np.concatenate(pieces, axis=0)


# revision 64
# speedup vs baseline: 1.0236x; 1.0059x over previous
"""BinaryTreeLSTM on 8 Trainium2 NeuronCores (Bass/Tile).

Sharding: each core owns a contiguous subtree of 4096 leaves and reduces
it through the internal levels down to 1024 nodes per core (87.5% of
FLOPs, fully pipelined on device with no flushes). The remaining global
levels (8192 nodes down to the root) are latency-bound on device —
each costs a serial matmul+activation chain plus a pipeline flush for a
shrinking node count — so they are finished on the host with BLAS from
the per-core level-1024 h/c. This also removes the AllGather collective
entirely. Gate weights are replicated across cores.

Per-level node arrays are stored in bit-reversed node order on device so
each level's left/right children are the contiguous halves of the child
level; the host un-permutes when reassembling the natural level-order
output.

Device layout: hidden dim (256 = 2 chunks of 128) on the SBUF partition
axis, nodes on the free axis. Matmuls default to bf16 operands (fp32
PSUM accumulation; ~1e-3 output error) which halves input DMA and
enables fast weight loads; set TRNK_MM_DTYPE=float32r for the fp32r
path. Gate math is fp32 on the Scalar/Vector engines with the per-gate
bias folded into the activation instruction. Inputs stream in over both
hardware-DGE queues (Scalar + Sync) so descriptor generation for the
first-needed pieces overlaps.
"""

import os
import sys

import numpy as np

sys.path.insert(0, "/opt/trn_rl_repo")

HIDDEN = 256
NCORES = 8
CH = 512        # node-chunk (PSUM bank / fp32 moving-operand limit)
CUT = 1024      # smallest per-core level computed on device

# exposed for test harnesses
LAST_RESULTS = None
LAST_EXEC_NS = None
LAST_OPS = None


def _revperm(n):
    bits = n.bit_length() - 1
    r = np.arange(n)
    out = np.zeros(n, np.int64)
    for b in range(bits):
        out |= ((r >> b) & 1) << (bits - 1 - b)
    return out


def _w_tile_index(src, g, kc, hc):
    return ((src * 4 + g) * 2 + kc) * 2 + hc


def _round_fp32r(a):
    """Round fp32 values to the PE's fp32r format (1+8+11 bits, RNE)."""
    bits = np.ascontiguousarray(a, np.float32).view(np.uint32)
    odd = (bits >> np.uint32(12)) & np.uint32(1)
    bits = bits + np.uint32(0x7FF) + odd
    bits &= np.uint32(0xFFFFF000)
    return bits.view(np.float32)


def _pack_weights(Wx, Wl, Wr):
    # lhsT tile for (src, g, kc, hc): [p(contraction), m(out)] = W[g, hc*128+m, kc*128+p]
    tiles = []
    for W in (Wx, Wl, Wr):
        W4 = W.reshape(4, 2, 128, 2, 128)           # [g, hc, m, kc, p]
        tiles.append(W4.transpose(0, 3, 1, 4, 2))    # [g, kc, hc, p, m]
    allw = np.stack(tiles)                            # [3, 4, 2, 2, 128, 128]
    # -> [p, (s,g,kc,hc), m]
    blob = np.ascontiguousarray(allw.transpose(4, 0, 1, 2, 3, 5).reshape(128, 48, 128))
    return blob.astype(np.float32)


def _build_program(LPC, matmul_dtype_name="bfloat16"):
    from concourse import bacc, mybir, tile

    f32 = mybir.dt.float32
    mmdt = getattr(mybir.dt, matmul_dtype_name)
    bf16 = matmul_dtype_name == "bfloat16"
    sdt = mmdt if bf16 else f32
    AF = mybir.ActivationFunctionType

    sizes = []
    n = LPC
    while n >= CUT:
        sizes.append(n)
        n //= 2
    offs = np.concatenate([[0], np.cumsum(sizes)]).astype(int)
    TOT = int(offs[-1])
    NCHUNK = LPC // CH

    nc = bacc.Bacc("TRN2", target_bir_lowering=False, debug=False,
                   num_devices=NCORES)

    x_d = nc.dram_tensor("x", [128, NCHUNK, 2, CH], mmdt,
                         kind="ExternalInput").ap()
    wt_d = nc.dram_tensor("wt", [128, 48, 128], mmdt, kind="ExternalInput").ap()
    bias_d = nc.dram_tensor("bias", [128, 8], f32, kind="ExternalInput").ap()
    out_d = nc.dram_tensor("out", [2, 128, TOT], sdt, kind="ExternalOutput").ap()
    cend_d = nc.dram_tensor("cend", [2, 128, CUT], f32, kind="ExternalOutput").ap()

    with tile.TileContext(nc) as tc:
        with tc.tile_pool(name="pp", bufs=1) as pp, \
             tc.tile_pool(name="zp", bufs=6, space="PSUM") as zp, \
             tc.tile_pool(name="gp", bufs=3) as gp:
            w_sb = pp.tile([128, 48, 128], mmdt, name="w_sb")
            bias_sb = pp.tile([128, 8], f32, name="bias_sb")
            hA = pp.tile([128, 2, LPC], sdt, name="hA")
            cA = pp.tile([128, 2, LPC], f32, name="cA")
            hB = pp.tile([128, 2, LPC // 2], sdt, name="hB")
            cB = pp.tile([128, 2, LPC // 2], f32, name="cB")
            x_sb = [pp.tile([128, 2, CH], mmdt, name=f"x_sb{ch}")
                    for ch in range(NCHUNK)]

            # Inputs split across both HW-DGE queues (one queue per engine —
            # same-engine DMAs serialize): bias + leaf weights on Scalar,
            # x chunks on Sync, so the first matmul's inputs land early.
            nc.scalar.dma_start(out=bias_sb[:], in_=bias_d[:])
            nc.scalar.dma_start(out=w_sb[:, 0:16, :], in_=wt_d[:, 0:16, :])
            for ch in (2, 5):
                nc.scalar.dma_start(out=x_sb[ch][:], in_=x_d[:, ch])
            nc.scalar.dma_start(out=w_sb[:, 16:48, :], in_=wt_d[:, 16:48, :])
            for ch in (0, 1, 3, 4, 6, 7):
                nc.sync.dma_start(out=x_sb[ch][:], in_=x_d[:, ch])

            if bf16:
                cast_rhs = lambda ap: ap  # noqa: E731
            else:
                cast_rhs = lambda ap: ap.bitcast(mmdt)  # noqa: E731

            def mm(w_idx, rhs_ap, zt, start, stop):
                nc.tensor.matmul(zt, w_sb[:, w_idx, :], cast_rhs(rhs_ap),
                                 start=start, stop=stop)

            def unit_internal(n, ch, h_src, c_src, h_dst, c_dst, out_off,
                              c_out=False):
                """One chunk of an internal level -> (stage1, stage2)."""
                nchunks = max(1, n // CH)
                m = min(n, CH)
                lsl = slice(ch * m, (ch + 1) * m)
                rsl = slice(n + ch * m, n + (ch + 1) * m)
                dsl = slice(ch * m, (ch + 1) * m)
                i_t = gp.tile([128, 2, CH], f32, name="i_t")
                f_t = gp.tile([128, 2, CH], f32, name="f_t")
                o_t = gp.tile([128, 2, CH], f32, name="o_t")
                u_t = gp.tile([128, 2, CH], f32, name="u_t")
                s_t = gp.tile([128, 2, CH], f32, name="s_t")
                gates = {0: i_t, 1: f_t, 2: o_t, 3: u_t}

                def s1():
                    for hc in range(2):
                        nc.gpsimd.tensor_add(s_t[:, hc, :m],
                                             c_src[:, hc, lsl],
                                             c_src[:, hc, rsl])
                    for hc in range(2):
                        for g in (0, 3, 1, 2):
                            zt = zp.tile([128, CH], f32, name="zt")
                            mm(_w_tile_index(1, g, 0, hc), h_src[:, 0, lsl],
                               zt[:, :m], True, False)
                            mm(_w_tile_index(1, g, 1, hc), h_src[:, 1, lsl],
                               zt[:, :m], False, False)
                            mm(_w_tile_index(2, g, 0, hc), h_src[:, 0, rsl],
                               zt[:, :m], False, False)
                            mm(_w_tile_index(2, g, 1, hc), h_src[:, 1, rsl],
                               zt[:, :m], False, True)
                            func = AF.Tanh if g == 3 else AF.Sigmoid
                            nc.scalar.activation(
                                out=gates[g][:, hc, :m], in_=zt[:, :m],
                                func=func,
                                bias=bias_sb[:, g * 2 + hc:g * 2 + hc + 1])

                def s2():
                    # DVE ops stay per-hc (short chains); only tanh(c) is
                    # hc-merged — one 2m-column instruction instead of two,
                    # shaving per-instruction overhead off the saturated
                    # Scalar engine
                    for hc in range(2):
                        nc.vector.tensor_mul(u_t[:, hc, :m], i_t[:, hc, :m],
                                             u_t[:, hc, :m])
                        nc.vector.tensor_mul(s_t[:, hc, :m], f_t[:, hc, :m],
                                             s_t[:, hc, :m])
                        nc.vector.tensor_add(c_dst[:, hc, dsl],
                                             u_t[:, hc, :m], s_t[:, hc, :m])
                    nc.scalar.activation(out=i_t[:, :, :m],
                                         in_=c_dst[:, :, dsl], func=AF.Tanh)
                    for hc in range(2):
                        nc.vector.tensor_mul(h_dst[:, hc, dsl],
                                             o_t[:, hc, :m], i_t[:, hc, :m])
                    if ch == nchunks - 1:
                        for hc in range(2):
                            nc.sync.dma_start(
                                out=out_d[hc, :, out_off:out_off + n],
                                in_=h_dst[:, hc, :n])
                        if c_out:
                            for hc in range(2):
                                nc.sync.dma_start(
                                    out=cend_d[hc, :, :],
                                    in_=c_dst[:, hc, :n])
                return s1, s2

            def unit_leaves(ch):
                nsl = slice(ch * CH, (ch + 1) * CH)
                xc_t = x_sb[ch]
                i_t = gp.tile([128, 2, CH], f32, name="i_t")
                o_t = gp.tile([128, 2, CH], f32, name="o_t")
                u_t = gp.tile([128, 2, CH], f32, name="u_t")
                lgates = {0: i_t, 2: o_t, 3: u_t}

                def s1():
                    for hc in range(2):
                        for g in (0, 3, 2):
                            zt = zp.tile([128, CH], f32, name="zt")
                            mm(_w_tile_index(0, g, 0, hc), xc_t[:, 0, :],
                               zt[:], True, False)
                            mm(_w_tile_index(0, g, 1, hc), xc_t[:, 1, :],
                               zt[:], False, True)
                            func = AF.Tanh if g == 3 else AF.Sigmoid
                            nc.scalar.activation(
                                out=lgates[g][:, hc, :], in_=zt[:], func=func,
                                bias=bias_sb[:, g * 2 + hc:g * 2 + hc + 1])

                def s2():
                    for hc in range(2):
                        nc.vector.tensor_mul(cA[:, hc, nsl], i_t[:, hc, :],
                                             u_t[:, hc, :])
                    nc.scalar.activation(out=u_t[:], in_=cA[:, :, nsl],
                                         func=AF.Tanh)
                    for hc in range(2):
                        nc.vector.tensor_mul(hA[:, hc, nsl], o_t[:, hc, :],
                                             u_t[:, hc, :])
                    if ch == NCHUNK - 1:
                        for hc in range(2):
                            nc.sync.dma_start(out=out_d[hc, :, 0:LPC],
                                              in_=hA[:, hc, :])
                return s1, s2

            # ---- software-pipelined unit stream: leaves + internal levels ----
            units = [("leaf", ch, False) for ch in range(NCHUNK)]
            cur = [hA, cA, hB, cB]
            lvl = 1
            n = LPC // 2
            while n >= CUT:
                h_src, c_src, h_dst, c_dst = cur
                # if the child level had <= 2 chunks, this level's first s1
                # reads h written by a pending s2 -> must flush the pipeline
                flush = (2 * n) // CH <= 2
                for ch in range(max(1, n // CH)):
                    units.append(("int", (n, ch, h_src, c_src, h_dst, c_dst,
                                          int(offs[lvl]), n == CUT),
                                  flush and ch == 0))
                cur = [cur[2], cur[3], cur[0], cur[1]]
                lvl += 1
                n //= 2
            pending = []
            for u in units:
                if u[2]:
                    while pending:
                        pending.pop(0)()
                s1, s2 = (unit_leaves(u[1]) if u[0] == "leaf"
                          else unit_internal(*u[1]))
                s1()
                pending.append(s2)
                if len(pending) > 1:
                    pending.pop(0)()
            for s2 in pending:
                s2()

    nc.compile()
    return nc, sizes, offs, TOT


class _ExecHandle:
    """Compiled SPMD executable with device-resident input support."""

    def __init__(self, nc):
        import jax
        from jax.sharding import Mesh, PartitionSpec
        try:
            from jax.experimental.shard_map import shard_map
        except ImportError:
            from jax.shard_map import shard_map
        from concourse import bass2jax, mybir

        bass2jax.install_neuronx_cc_hook()
        self.jax = jax
        partition_name = (nc.partition_id_tensor.name
                          if nc.partition_id_tensor else None)
        in_names, out_names, out_avals, zero_outs = [], [], [], []
        for alloc in nc.m.functions[0].allocations:
            if not isinstance(alloc, mybir.MemoryLocationSet):
                continue
            name = alloc.memorylocations[0].name
            if alloc.kind == "ExternalInput":
                if name != partition_name:
                    in_names.append(name)
            elif alloc.kind == "ExternalOutput":
                out_names.append(name)
                shape = tuple(alloc.tensor_shape)
                dtype = mybir.dt.np(alloc.dtype)
                out_avals.append(jax.core.ShapedArray(shape, dtype))
                zero_outs.append(np.zeros(shape, dtype))
        self.n_params = len(in_names)
        self.out_names = list(out_names)
        self.param_names = list(in_names)
        all_in_names = in_names + out_names
        if partition_name is not None:
            all_in_names.append(partition_name)
        self.out_avals = out_avals
        self.zero_outs = zero_outs

        def _body(*args):
            operands = list(args)
            if partition_name is not None:
                operands.append(bass2jax.partition_id_tensor())
            outs = bass2jax._bass_exec_p.bind(
                *operands,
                out_avals=tuple(out_avals),
                in_names=tuple(all_in_names),
                out_names=tuple(out_names),
                lowering_input_output_aliases=(),
                sim_require_finite=True,
                sim_require_nnan=True,
                nc=nc,
            )
            return tuple(outs)

        self._body = _body

        devices = jax.devices()[:NCORES]
        self.mesh = Mesh(np.asarray(devices), ("core",))
        n_ops = self.n_params + len(out_names)
        self.fn = jax.jit(shard_map(
            _body, mesh=self.mesh,
            in_specs=(PartitionSpec("core"),) * n_ops,
            out_specs=(PartitionSpec("core"),) * len(out_names),
            check_rep=False))

    def put_inputs(self, in_maps):
        import jax
        from jax.sharding import NamedSharding, PartitionSpec
        sh = NamedSharding(self.mesh, PartitionSpec("core"))
        ops = []
        for i, name in enumerate(self.param_names):
            arr = np.concatenate([np.asarray(m[name]) for m in in_maps], axis=0)
            ops.append(jax.device_put(arr, sh))
        for z in self.zero_outs:
            zz = np.zeros((NCORES * z.shape[0], *z.shape[1:]), z.dtype)
            ops.append(jax.device_put(zz, sh))
        return ops

    def run(self, ops):
        outs = self.fn(*ops)
        self.jax.block_until_ready(outs)
        return outs

    def results(self, outs):
        res = []
        for c in range(NCORES):
            d = {}
            for i, name in enumerate(self.out_names):
                a = np.asarray(outs[i])
                d[name] = a.reshape(NCORES, *self.out_avals[i].shape)[c]
            res.append(d)
        return res


def _sigmoid(z):
    with np.errstate(over="ignore"):
        return 1.0 / (1.0 + np.exp(-z))


_PROGRAM_CACHE = {}
_EXEC_CACHE = {}


def kernel(tokens, emb, Wx, Wl, Wr, b):
    global LAST_RESULTS, LAST_OPS
    tokens = np.asarray(tokens)
    emb = np.asarray(emb, dtype=np.float32)
    Wx = np.asarray(Wx, dtype=np.float32)
    Wl = np.asarray(Wl, dtype=np.float32)
    Wr = np.asarray(Wr, dtype=np.float32)
    b = np.asarray(b, dtype=np.float32)

    L = int(tokens.shape[0])
    LPC = L // NCORES
    mmdt = os.environ.get("TRNK_MM_DTYPE", "bfloat16")
    key = (LPC, mmdt)
    if key not in _PROGRAM_CACHE:
        _PROGRAM_CACHE[key] = _build_program(LPC, mmdt)
    nc, sizes, offs, TOT = _PROGRAM_CACHE[key]

    wt_blob = _pack_weights(Wx, Wl, Wr)
    bias_blob = np.ascontiguousarray(
        b.reshape(4, 2, 128).transpose(2, 0, 1).reshape(128, 8)).astype(np.float32)

    x = emb[tokens]  # [L, 256] host gather (input sharding/staging)
    rp = _revperm(LPC)
    if mmdt == "bfloat16":
        import ml_dtypes
        wt_blob = wt_blob.astype(ml_dtypes.bfloat16)
        cast = lambda a: a.astype(ml_dtypes.bfloat16)  # noqa: E731
    else:
        wt_blob = _round_fp32r(wt_blob)
        cast = _round_fp32r
    in_maps = []
    for ci in range(NCORES):
        xc = x[ci * LPC:(ci + 1) * LPC][rp]                   # stored order
        # [128, NCHUNK, 2, CH]: chunk-major so each chunk DMA is 2KB lines
        xblob = np.ascontiguousarray(
            xc.reshape(LPC // CH, CH, 2, 128).transpose(3, 0, 2, 1))
        in_maps.append({"x": cast(xblob), "wt": wt_blob, "bias": bias_blob})

    if key not in _EXEC_CACHE:
        _EXEC_CACHE[key] = _ExecHandle(nc)
    eh = _EXEC_CACHE[key]
    ops = eh.put_inputs(in_maps)
    outs = eh.run(ops)
    results = eh.results(outs)
    LAST_RESULTS = results
    LAST_OPS = ops

    # ---- host reassembly of device levels (global 32768 .. 2048) ----
    pieces = []
    for lvl, npc in enumerate(sizes):
        nglob = npc * NCORES
        rpl = _revperm(npc)
        lvlarr = np.empty((nglob, HIDDEN), np.float32)
        for ci in range(NCORES):
            o = results[ci]["out"]                      # [2, 128, TOT]
            st = o[:, :, offs[lvl]:offs[lvl] + npc].reshape(HIDDEN, npc)
            lvlarr[ci * npc:(ci + 1) * npc] = st.T[rpl].astype(np.float32)
        pieces.append(lvlarr)

    # ---- host tail: global levels 1024 .. 1 from per-core (h,c) at CUT ----
    rpc = _revperm(CUT)
    nglob = CUT * NCORES
    h = np.empty((nglob, HIDDEN), np.float32)
    c = np.empty((nglob, HIDDEN), np.float32)
    for ci in range(NCORES):
        st = results[ci]["out"][:, :, offs[-2]:offs[-2] + CUT]
        h[ci * CUT:(ci + 1) * CUT] = st.reshape(HIDDEN, CUT).T[rpc].astype(np.float32)
        stc = results[ci]["cend"]                       # [2, 128, CUT] f32
        c[ci * CUT:(ci + 1) * CUT] = stc.reshape(HIDDEN, CUT).T[rpc]

    # y = x @ W.T per gate; stack gates on columns: [in, 4*out]
    WlT = np.ascontiguousarray(Wl.transpose(2, 0, 1).reshape(HIDDEN, 4 * HIDDEN))
    WrT = np.ascontiguousarray(Wr.transpose(2, 0, 1).reshape(HIDDEN, 4 * HIDDEN))
    bfl = b.reshape(4 * HIDDEN)
    while h.shape[0] > 1:
        lh, rh = h[0::2], h[1::2]
        lc, rc = c[0::2], c[1::2]
        z = lh @ WlT + rh @ WrT + bfl                   # [n, 4H]
        i = _sigmoid(z[:, 0 * HIDDEN:1 * HIDDEN])
        f = _sigmoid(z[:, 1 * HIDDEN:2 * HIDDEN])
        o = _sigmoid(z[:, 2 * HIDDEN:3 * HIDDEN])
        u = np.tanh(z[:, 3 * HIDDEN:4 * HIDDEN])
        c = i * u + f * (lc + rc)
        h = o * np.tanh(c)
        pieces.append(h)
    return np.concatenate(pieces, axis=0)
